# revision 13
# baseline (speedup 1.0000x reference)
"""Conditional VQ embedding forward on 8 trn2 NeuronCores.

Data-parallel over batch: 4 batches per core. Per batch b, per n-tile of 128
positions:
  s[n,k]  = z[b,n,:] . cb[b,k,:] via 3-pass bf16 hi/lo split matmuls
            (zh.eh + zh.el + zl.eh, fp32 PSUM accumulate). Reproduces the
            fp32 score to ~1e-9 - far below the reference's fp32 rounding
            grid, so argmin picks match the reference (verified: <=1 flip
            of 131072 in simulation).
  v[n,k]  = fp32(2*s - ||z_n||^2)  (ACT Identity, per-partition bias) -
            replicates the reference's fp32 rounding of the distance, whose
            ~2^-15 quantization grid creates index ties that are load-bearing
            (~2% of picks).
  idx[n]  = argmax_k v, first index on ties (DVE max8 + max_index).
  quant   = cb[b, idx[n], :] gathered IN TRANSPOSED [d, n] layout by a GPSIMD
            ap_gather from an SBUF-resident codebook table - no PE transposes,
            no HBM gather traffic.
Output z_q_x == z_q_x_bar == quant numerically (the straight-through forward
value z + fp32(quant - z) differs from quant by ~ulp(z) ~ 1e-3 relative -
far inside the 2e-2 tolerance), so the device writes one output and the host
returns it for both.
"""

import numpy as np

B, D, HW, K = 32, 256, 4096, 512
NCORES, BPC = 8, 4
P = 128
NT = HW // P  # 32 n-tiles of 128 per batch

GW = 1024  # n-columns per group (8 tiles)
NG = HW // GW  # 4 groups per batch
TPG = GW // P  # 8 tiles per group

TRACE = False
DEBUG_MEMSET = False
LAST_RESULT = None
_NC_CACHE = {}


def _build():
    from contextlib import ExitStack

    import concourse.bass as bass
    import concourse.mybir as mybir
    from concourse import bacc
    from concourse.tile import TileContext

    f32 = mybir.dt.float32
    bf16 = mybir.dt.bfloat16
    u16 = mybir.dt.uint16
    i16 = mybir.dt.int16

    nc = bacc.Bacc("TRN2", target_bir_lowering=False, debug=False, num_devices=NCORES)
    zh_in = nc.dram_tensor("zh", [BPC, D, HW], bf16, kind="ExternalInput")
    zl_in = nc.dram_tensor("zl", [BPC, D, HW], bf16, kind="ExternalInput")
    ch_in = nc.dram_tensor("chT", [BPC, D, K], bf16, kind="ExternalInput")
    cl_in = nc.dram_tensor("clT", [BPC, D, K], bf16, kind="ExternalInput")
    cbg_in = nc.dram_tensor("cbg", [BPC, 2, P, K], f32, kind="ExternalInput")
    an_in = nc.dram_tensor("an", [BPC, HW], f32, kind="ExternalInput")
    q_out = nc.dram_tensor("q", [BPC, 2, P, HW], f32, kind="ExternalOutput")

    with TileContext(nc) as tc, ExitStack() as ctx:
        cb_p = ctx.enter_context(tc.tile_pool(name="cbp", bufs=2))
        an_p = ctx.enter_context(tc.tile_pool(name="anp", bufs=2))
        z_p = ctx.enter_context(tc.tile_pool(name="zp", bufs=3))
        v_p = ctx.enter_context(tc.tile_pool(name="vp", bufs=4))
        m_p = ctx.enter_context(tc.tile_pool(name="mp", bufs=8))
        ix_p = ctx.enter_context(tc.tile_pool(name="ixp", bufs=3))
        qg_p = ctx.enter_context(tc.tile_pool(name="qgp", bufs=2))
        ps_p = ctx.enter_context(tc.tile_pool(name="psp", bufs=6, space="PSUM"))

        for b in range(BPC):
            ch0 = cb_p.tile([P, K], bf16, tag="ch0")
            ch1 = cb_p.tile([P, K], bf16, tag="ch1")
            cl0 = cb_p.tile([P, K], bf16, tag="cl0")
            cl1 = cb_p.tile([P, K], bf16, tag="cl1")
            nc.sync.dma_start(ch0[:], ch_in[b, 0:P, :])
            nc.sync.dma_start(ch1[:], ch_in[b, P : 2 * P, :])
            nc.sync.dma_start(cl0[:], cl_in[b, 0:P, :])
            nc.sync.dma_start(cl1[:], cl_in[b, P : 2 * P, :])
            cbg0 = cb_p.tile([P, K], f32, tag="cbg0")
            cbg1 = cb_p.tile([P, K], f32, tag="cbg1")
            nc.sync.dma_start(cbg0[:], cbg_in[b, 0])
            nc.sync.dma_start(cbg1[:], cbg_in[b, 1])
            an_all = an_p.tile([P, NT], f32, tag="an")
            nc.sync.dma_start(an_all[:], an_in[b, :].rearrange("(t p) -> p t", p=P))

            i8c = m_p.tile([P, NT], u16, tag="i8c")
            for g in range(NG):
                gs = slice(g * GW, (g + 1) * GW)
                zh_g = z_p.tile([P, 2, GW], bf16, tag="zh")
                zl_g = z_p.tile([P, 2, GW], bf16, tag="zl")
                nc.sync.dma_start(zh_g[:], zh_in[b, :, gs].rearrange("(c p) n -> p c n", p=P))
                nc.sync.dma_start(zl_g[:], zl_in[b, :, gs].rearrange("(c p) n -> p c n", p=P))

                i8g = m_p.tile([P, TPG, 8], u16, tag="i8g")
                for u in range(TPG):
                    t = g * TPG + u
                    us = slice(u * P, (u + 1) * P)

                    ps = ps_p.tile([P, K], f32, space="PSUM", tag="ps")
                    nc.tensor.matmul(ps[:], lhsT=zh_g[:, 0, us], rhs=ch0[:], start=True, stop=False)
                    nc.tensor.matmul(ps[:], lhsT=zh_g[:, 0, us], rhs=cl0[:], start=False, stop=False)
                    nc.tensor.matmul(ps[:], lhsT=zl_g[:, 0, us], rhs=ch0[:], start=False, stop=False)
                    nc.tensor.matmul(ps[:], lhsT=zh_g[:, 1, us], rhs=ch1[:], start=False, stop=False)
                    nc.tensor.matmul(ps[:], lhsT=zh_g[:, 1, us], rhs=cl1[:], start=False, stop=False)
                    nc.tensor.matmul(ps[:], lhsT=zl_g[:, 1, us], rhs=ch1[:], start=False, stop=True)

                    v = v_p.tile([P, K], f32, tag="v")
                    nc.scalar.activation(
                        out=v[:], in_=ps[:],
                        func=mybir.ActivationFunctionType.Identity,
                        bias=an_all[:, t : t + 1], scale=2.0,
                    )
                    m8 = m_p.tile([P, 8], f32, tag="m8")
                    nc.vector.max(out=m8[:], in_=v[:])
                    nc.vector.max_index(out=i8g[:, u, :], in_max=m8[:], in_values=v[:])

                nc.vector.tensor_copy(i8c[:, TPG * g : TPG * (g + 1)], i8g[:, :, 0])

            # Relayout the batch's 4096 argmax indices into ap_gather's
            # wrapped layout (partition j%16, column j//16, replicated per
            # 16-partition Q7 group) using plain partition-range DMAs. The
            # column order this produces equals a bit-field permutation of
            # the position index, which the host pre-applies (inverted) to
            # z/an so the gather output lands in identity order.
            idx16 = ix_p.tile([P, HW // 16], i16, tag="idx16")
            # These DMAs must NOT issue from the gpsimd queue: ap_gather
            # lives in a separate Q7 library from the SWDGE DMA ucode, and
            # interleaving them forces a ~113us library reload per batch.
            for s2 in range(8):
                nc.sync.dma_start(
                    out=idx16[0:16, NT * s2 : NT * (s2 + 1)].bitcast(u16),
                    in_=i8c[16 * s2 : 16 * (s2 + 1), :],
                )
            nc.sync.dma_start(out=idx16[16:32, :], in_=idx16[0:16, :])
            nc.sync.dma_start(out=idx16[32:64, :], in_=idx16[0:32, :])
            nc.sync.dma_start(out=idx16[64:128, :], in_=idx16[0:64, :])

            for c, cbgc in ((0, cbg0), (1, cbg1)):
                qg = qg_p.tile([P, HW], f32, tag=f"qg{c}")
                nc.gpsimd.ap_gather(
                    out_ap=qg[:],
                    in_ap=cbgc[:],
                    idxs_ap=idx16[:],
                    channels=P,
                    num_elems=K,
                    d=1,
                    num_idxs=HW,
                )
                nc.sync.dma_start(out=q_out[b, c, :, :], in_=qg[:])

    nc.compile()
    return nc


def _get_nc():
    if "nc" not in _NC_CACHE:
        _NC_CACHE["nc"] = _build()
    return _NC_CACHE["nc"]


def kernel(z_e_x, C, weight):
    global LAST_RESULT
    import ml_dtypes
    from concourse.bass_utils import run_bass_kernel_spmd

    z_e_x = np.asarray(z_e_x, dtype=np.float32)
    C = np.asarray(C).astype(np.int64)
    weight = np.asarray(weight, dtype=np.float32)

    # ||z_n||^2 computed with the exact op sequence of the reference on the
    # default jax backend, so the fp32 bits match the reference's dist term.
    import jax.numpy as jnp

    zj = jnp.asarray(z_e_x)
    zr = jnp.transpose(zj, (0, 2, 3, 1)).reshape(B, HW, D)
    A = jnp.sum(zr * zr, axis=-1, keepdims=True)
    an = -np.asarray(A)[..., 0]  # [B, HW] fp32, negated for the ACT bias
    # (permuted below together with z)

    # Position permutation: gather column j reads the index computed for
    # processing slot s(j) = (bits[4:9)(j) << 7) | (bits[9:12)(j) << 4) |
    # bits[0:4)(j); loading slot s with actual position perm[s] =
    # (bits[4:7)(s) << 9) | (bits[7:12)(s) << 4) | bits[0:4)(s) makes
    # column j hold position j exactly.
    r = np.arange(HW)
    perm = (((r >> 4) & 7) << 9) | (((r >> 7) & 31) << 4) | (r & 15)

    an = an[:, perm]
    zflat = z_e_x.reshape(B, D, HW)[:, :, perm]
    zh = zflat.astype(ml_dtypes.bfloat16)
    zl = (zflat - zh.astype(np.float32)).astype(ml_dtypes.bfloat16)

    cb_all = weight[C]  # [B, K, D] fp32
    ch = cb_all.astype(ml_dtypes.bfloat16)
    cl = (cb_all - ch.astype(np.float32)).astype(ml_dtypes.bfloat16)
    chT = np.ascontiguousarray(np.swapaxes(ch, 1, 2))  # [B, D, K] bf16
    clT = np.ascontiguousarray(np.swapaxes(cl, 1, 2))
    # gather table: cbg[b, c, p, k] = cb[b, k, 128c + p]
    cbg = np.ascontiguousarray(cb_all.reshape(B, K, 2, P).transpose(0, 2, 3, 1))

    nc = _get_nc()
    in_maps = []
    for c in range(NCORES):
        bs = slice(c * BPC, (c + 1) * BPC)
        in_maps.append(
            dict(
                zh=np.ascontiguousarray(zh[bs]),
                zl=np.ascontiguousarray(zl[bs]),
                chT=chT[bs],
                clT=clT[bs],
                cbg=cbg[bs],
                an=np.ascontiguousarray(an[bs]).astype(np.float32),
            )
        )
    res = run_bass_kernel_spmd(nc, in_maps, core_ids=list(range(NCORES)), trace=TRACE)
    LAST_RESULT = res
    q = np.concatenate([r["q"] for r in res.results], 0).reshape(B, D, 64, 64)
    return q, q  # (z_q_x, z_q_x_bar) - numerically identical within fp32 noise


# revision 15
# speedup vs baseline: 2.6885x; 2.6885x over previous
"""Conditional VQ embedding forward on 8 trn2 NeuronCores.

Data-parallel over batch: 4 batches per core. Per batch b, per n-tile of 128
positions:
  s[n,k]  = z[b,n,:] . cb[b,k,:] via 3-pass bf16 hi/lo split matmuls
            (zh.eh + zh.el + zl.eh, fp32 PSUM accumulate). Reproduces the
            fp32 score to ~1e-9 - far below the reference's fp32 rounding
            grid, so argmin picks match the reference (verified: 1 flip
            of 131072 positions).
  v[n,k]  = fp32(2*s - ||z_n||^2)  (ACT Identity, per-partition bias) -
            replicates the reference's fp32 rounding of the distance, whose
            ~2^-15 quantization grid creates index ties that are load-bearing
            (~2% of picks).
  idx[n]  = argmax_k v, first index on ties (DVE max8 + max_index).
  quant   = cb[b, idx[n], :] via SWDGE indirect DMA row gather (the only op
            on the gpsimd queue - custom Q7 ops would force library reloads),
            then PE-transposed to [d, n] layout and evacuated by ACT.
The backend (gather/transpose/evac) for tile t is emitted LAG tiles behind
the frontend so the PE/ACT queues never head-block on the gather latency.
Output z_q_x == z_q_x_bar == quant numerically (the straight-through forward
value z + fp32(quant - z) differs from quant by ~ulp(z) ~ 1e-3 relative -
far inside the 2e-2 tolerance), so the device writes one output and the host
returns it for both.
"""

import numpy as np

B, D, HW, K = 32, 256, 4096, 512
NCORES, BPC = 8, 4
P = 128
NT = HW // P  # 32 n-tiles of 128 per batch

GW = 1024  # n-columns per output group (8 tiles)
NG = HW // GW
TPG = GW // P

LAG = 3  # tiles of backend lag to hide the gather latency

TRACE = False
LAST_RESULT = None
_NC_CACHE = {}


def _build():
    from contextlib import ExitStack

    import concourse.bass as bass
    import concourse.mybir as mybir
    from concourse import bacc
    from concourse.tile import TileContext
    from concourse.masks import make_identity

    f32 = mybir.dt.float32
    bf16 = mybir.dt.bfloat16
    u32 = mybir.dt.uint32

    nc = bacc.Bacc("TRN2", target_bir_lowering=False, debug=False, num_devices=NCORES)
    zh_in = nc.dram_tensor("zh", [BPC, D, HW], bf16, kind="ExternalInput")
    zl_in = nc.dram_tensor("zl", [BPC, D, HW], bf16, kind="ExternalInput")
    ch_in = nc.dram_tensor("chT", [BPC, D, K], bf16, kind="ExternalInput")
    cl_in = nc.dram_tensor("clT", [BPC, D, K], bf16, kind="ExternalInput")
    cb_in = nc.dram_tensor("cb", [BPC * K, D], f32, kind="ExternalInput")
    an_in = nc.dram_tensor("an", [BPC, HW], f32, kind="ExternalInput")
    q_out = nc.dram_tensor("q", [BPC, D, HW], f32, kind="ExternalOutput")

    with TileContext(nc) as tc, ExitStack() as ctx:
        const_p = ctx.enter_context(tc.tile_pool(name="const", bufs=1))
        cb_p = ctx.enter_context(tc.tile_pool(name="cbp", bufs=2))
        an_p = ctx.enter_context(tc.tile_pool(name="anp", bufs=2))
        z_p = ctx.enter_context(tc.tile_pool(name="zp", bufs=3))
        v_p = ctx.enter_context(tc.tile_pool(name="vp", bufs=4))
        m_p = ctx.enter_context(tc.tile_pool(name="mp", bufs=8))
        qu_p = ctx.enter_context(tc.tile_pool(name="qup", bufs=8))
        w_p = ctx.enter_context(tc.tile_pool(name="wp", bufs=2))
        ps_p = ctx.enter_context(tc.tile_pool(name="psp", bufs=4, space="PSUM"))
        pt_p = ctx.enter_context(tc.tile_pool(name="ptp", bufs=4, space="PSUM"))

        ident = const_p.tile([P, P], f32)
        make_identity(nc, ident[:])

        for b in range(BPC):
            ch0 = cb_p.tile([P, K], bf16, tag="ch0")
            ch1 = cb_p.tile([P, K], bf16, tag="ch1")
            cl0 = cb_p.tile([P, K], bf16, tag="cl0")
            cl1 = cb_p.tile([P, K], bf16, tag="cl1")
            nc.sync.dma_start(ch0[:], ch_in[b, 0:P, :])
            nc.sync.dma_start(ch1[:], ch_in[b, P : 2 * P, :])
            nc.sync.dma_start(cl0[:], cl_in[b, 0:P, :])
            nc.sync.dma_start(cl1[:], cl_in[b, P : 2 * P, :])
            an_all = an_p.tile([P, NT], f32, tag="an")
            nc.sync.dma_start(an_all[:], an_in[b, :].rearrange("(t p) -> p t", p=P))

            zh_gs, zl_gs, qu_ts, qtw_gs = {}, {}, {}, {}

            def frontend(t):
                g, u = t // TPG, t % TPG
                if u == 0:
                    gs = slice(g * GW, (g + 1) * GW)
                    zh_g = z_p.tile([P, 2, GW], bf16, tag="zh")
                    zl_g = z_p.tile([P, 2, GW], bf16, tag="zl")
                    nc.sync.dma_start(zh_g[:], zh_in[b, :, gs].rearrange("(c p) n -> p c n", p=P))
                    nc.sync.dma_start(zl_g[:], zl_in[b, :, gs].rearrange("(c p) n -> p c n", p=P))
                    zh_gs[g], zl_gs[g] = zh_g, zl_g
                zh_g, zl_g = zh_gs[g], zl_gs[g]
                us = slice(u * P, (u + 1) * P)

                ps = ps_p.tile([P, K], f32, space="PSUM", tag="ps")
                nc.tensor.matmul(ps[:], lhsT=zh_g[:, 0, us], rhs=ch0[:], start=True, stop=False)
                nc.tensor.matmul(ps[:], lhsT=zh_g[:, 0, us], rhs=cl0[:], start=False, stop=False)
                nc.tensor.matmul(ps[:], lhsT=zl_g[:, 0, us], rhs=ch0[:], start=False, stop=False)
                nc.tensor.matmul(ps[:], lhsT=zh_g[:, 1, us], rhs=ch1[:], start=False, stop=False)
                nc.tensor.matmul(ps[:], lhsT=zh_g[:, 1, us], rhs=cl1[:], start=False, stop=False)
                nc.tensor.matmul(ps[:], lhsT=zl_g[:, 1, us], rhs=ch1[:], start=False, stop=True)

                v = v_p.tile([P, K], f32, tag="v")
                nc.scalar.activation(
                    out=v[:], in_=ps[:],
                    func=mybir.ActivationFunctionType.Identity,
                    bias=an_all[:, t : t + 1], scale=2.0,
                )
                m8 = m_p.tile([P, 8], f32, tag="m8")
                nc.vector.max(out=m8[:], in_=v[:])
                i8 = m_p.tile([P, 8], u32, tag="i8")
                nc.vector.max_index(out=i8[:], in_max=m8[:], in_values=v[:])

                qu = qu_p.tile([P, 2 * P], f32, tag="qu")
                nc.gpsimd.indirect_dma_start(
                    out=qu[:],
                    out_offset=None,
                    in_=cb_in[:, :],
                    in_offset=bass.IndirectOffsetOnAxis(ap=i8[:, 0:1], axis=0),
                    element_offset=b * K * D,
                )
                qu_ts[t] = qu

            def backend(t):
                g, u = t // TPG, t % TPG
                if u == 0:
                    qtw = w_p.tile([P, 2, GW], f32, tag="qtw")
                    qtw_gs[g] = qtw
                qtw = qtw_gs[g]
                qu = qu_ts.pop(t)
                us = slice(u * P, (u + 1) * P)
                pst = pt_p.tile([P, 2, P], f32, space="PSUM", tag="pst")
                nc.tensor.matmul(pst[:, 0, :], lhsT=qu[:, 0:P], rhs=ident[:], is_transpose=True, start=True, stop=False)
                nc.tensor.matmul(pst[:, 1, :], lhsT=qu[:, P : 2 * P], rhs=ident[:], is_transpose=True, start=False, stop=False)
                nc.scalar.copy(out=qtw[:, :, us], in_=pst[:])
                if u == TPG - 1:
                    gs = slice(g * GW, (g + 1) * GW)
                    nc.sync.dma_start(q_out[b, :, gs].rearrange("(c p) n -> p c n", p=P), qtw[:])

            for t in range(NT):
                frontend(t)
                if t >= LAG:
                    backend(t - LAG)
            for t in range(NT - LAG, NT):
                backend(t)

    nc.compile()
    return nc


def _get_nc():
    if "nc" not in _NC_CACHE:
        _NC_CACHE["nc"] = _build()
    return _NC_CACHE["nc"]


def kernel(z_e_x, C, weight):
    global LAST_RESULT
    import ml_dtypes
    from concourse.bass_utils import run_bass_kernel_spmd

    z_e_x = np.asarray(z_e_x, dtype=np.float32)
    C = np.asarray(C).astype(np.int64)
    weight = np.asarray(weight, dtype=np.float32)

    # ||z_n||^2 computed with the exact op sequence of the reference on the
    # default jax backend, so the fp32 bits match the reference's dist term.
    import jax.numpy as jnp

    zj = jnp.asarray(z_e_x)
    zr = jnp.transpose(zj, (0, 2, 3, 1)).reshape(B, HW, D)
    A = jnp.sum(zr * zr, axis=-1, keepdims=True)
    an = -np.asarray(A)[..., 0]  # [B, HW] fp32, negated for the ACT bias

    zflat = z_e_x.reshape(B, D, HW)
    zh = zflat.astype(ml_dtypes.bfloat16)
    zl = (zflat - zh.astype(np.float32)).astype(ml_dtypes.bfloat16)

    cb_all = weight[C]  # [B, K, D] fp32
    ch = cb_all.astype(ml_dtypes.bfloat16)
    cl = (cb_all - ch.astype(np.float32)).astype(ml_dtypes.bfloat16)
    chT = np.ascontiguousarray(np.swapaxes(ch, 1, 2))  # [B, D, K] bf16
    clT = np.ascontiguousarray(np.swapaxes(cl, 1, 2))

    nc = _get_nc()
    in_maps = []
    for c in range(NCORES):
        bs = slice(c * BPC, (c + 1) * BPC)
        in_maps.append(
            dict(
                zh=np.ascontiguousarray(zh[bs]),
                zl=np.ascontiguousarray(zl[bs]),
                chT=chT[bs],
                clT=clT[bs],
                cb=np.ascontiguousarray(cb_all[bs].reshape(BPC * K, D)),
                an=np.ascontiguousarray(an[bs]).astype(np.float32),
            )
        )
    res = run_bass_kernel_spmd(nc, in_maps, core_ids=list(range(NCORES)), trace=TRACE)
    LAST_RESULT = res
    q = np.concatenate([r["q"] for r in res.results], 0).reshape(B, D, 64, 64)
    return q, q  # (z_q_x, z_q_x_bar) - numerically identical within fp32 noise


# revision 19
# speedup vs baseline: 4.9813x; 1.8529x over previous
"""Conditional VQ embedding forward on 8 trn2 NeuronCores.

Data-parallel over batch: 4 batches per core. Per batch b, per n-tile of 128
positions:
  s[n,k]  = z[b,n,:] . cb[b,k,:] via 3-pass bf16 hi/lo split matmuls
            (zh.eh + zh.el + zl.eh, fp32 PSUM accumulate). Reproduces the
            fp32 score to ~1e-9 - far below the reference's fp32 rounding
            grid, so argmin picks match the reference (verified: 1 flip
            of 131072 positions).
  v[n,k]  = fp32(2*s - ||z_n||^2)  (ACT Identity, per-partition bias) -
            replicates the reference's fp32 rounding of the distance, whose
            ~2^-15 quantization grid creates index ties that are load-bearing
            (~2% of picks).
  idx[n]  = argmax_k v, first index on ties (DVE max8 + max_index).
  quant   = cb[b, idx[n], :] via one SWDGE indirect DMA row gather per
            8-tile group from a bf16 codebook copy (the only op on the gpsimd
            queue - custom Q7 ops would force library reloads), written to
            DRAM in [n, d] row layout; the host reorders axes and upcasts
            when unsharding.
Output z_q_x == z_q_x_bar == quant numerically (the straight-through forward
value z + fp32(quant - z) differs from quant by ~ulp(z) ~ 1e-3 relative, and
bf16 rounding of the codewords ~1e-3 - both far inside the 2e-2 tolerance),
so the device writes one bf16 output and the host returns it for both.
"""

import numpy as np

B, D, HW, K = 32, 256, 4096, 512
NCORES, BPC = 8, 4
P = 128
NT = HW // P  # 32 n-tiles of 128 per batch

GW = 1024  # n-columns per output group (8 tiles)
NG = HW // GW
TPG = GW // P

TRACE = False
LAST_RESULT = None
_NC_CACHE = {}


def _build():
    from contextlib import ExitStack

    import concourse.bass as bass
    import concourse.mybir as mybir
    from concourse import bacc
    from concourse.tile import TileContext

    f32 = mybir.dt.float32
    bf16 = mybir.dt.bfloat16
    u32 = mybir.dt.uint32

    nc = bacc.Bacc("TRN2", target_bir_lowering=False, debug=False, num_devices=NCORES)
    zh_in = nc.dram_tensor("zh", [BPC, D, HW], bf16, kind="ExternalInput")
    zl_in = nc.dram_tensor("zl", [BPC, D, HW], bf16, kind="ExternalInput")
    ch_in = nc.dram_tensor("chT", [BPC, D, K], bf16, kind="ExternalInput")
    cl_in = nc.dram_tensor("clT", [BPC, D, K], bf16, kind="ExternalInput")
    cb_in = nc.dram_tensor("cb", [BPC * K, D], bf16, kind="ExternalInput")
    an_in = nc.dram_tensor("an", [BPC, HW], f32, kind="ExternalInput")
    q_out = nc.dram_tensor("q", [BPC, HW, D], bf16, kind="ExternalOutput")

    with TileContext(nc) as tc, ExitStack() as ctx:
        cb_p = ctx.enter_context(tc.tile_pool(name="cbp", bufs=2))
        an_p = ctx.enter_context(tc.tile_pool(name="anp", bufs=2))
        z_p = ctx.enter_context(tc.tile_pool(name="zp", bufs=3))
        v_p = ctx.enter_context(tc.tile_pool(name="vp", bufs=4))
        m_p = ctx.enter_context(tc.tile_pool(name="mp", bufs=8))
        qu_p = ctx.enter_context(tc.tile_pool(name="qup", bufs=3))
        ps_p = ctx.enter_context(tc.tile_pool(name="psp", bufs=6, space="PSUM"))

        for b in range(BPC):
            ch0 = cb_p.tile([P, K], bf16, tag="ch0")
            ch1 = cb_p.tile([P, K], bf16, tag="ch1")
            cl0 = cb_p.tile([P, K], bf16, tag="cl0")
            cl1 = cb_p.tile([P, K], bf16, tag="cl1")
            nc.sync.dma_start(ch0[:], ch_in[b, 0:P, :])
            nc.sync.dma_start(ch1[:], ch_in[b, P : 2 * P, :])
            nc.sync.dma_start(cl0[:], cl_in[b, 0:P, :])
            nc.sync.dma_start(cl1[:], cl_in[b, P : 2 * P, :])
            an_all = an_p.tile([P, NT], f32, tag="an")
            nc.sync.dma_start(an_all[:], an_in[b, :].rearrange("(t p) -> p t", p=P))

            for g in range(NG):
                gs = slice(g * GW, (g + 1) * GW)
                zh_g = z_p.tile([P, 2, GW], bf16, tag="zh")
                zl_g = z_p.tile([P, 2, GW], bf16, tag="zl")
                nc.sync.dma_start(zh_g[:], zh_in[b, :, gs].rearrange("(c p) n -> p c n", p=P))
                nc.sync.dma_start(zl_g[:], zl_in[b, :, gs].rearrange("(c p) n -> p c n", p=P))
                qu_g = qu_p.tile([P, TPG, D], bf16, tag="qug")
                for u in range(TPG):
                    t = g * TPG + u
                    us = slice(u * P, (u + 1) * P)

                    ps = ps_p.tile([P, K], f32, space="PSUM", tag="ps")
                    nc.tensor.matmul(ps[:], lhsT=zh_g[:, 0, us], rhs=ch0[:], start=True, stop=False)
                    nc.tensor.matmul(ps[:], lhsT=zh_g[:, 0, us], rhs=cl0[:], start=False, stop=False)
                    nc.tensor.matmul(ps[:], lhsT=zl_g[:, 0, us], rhs=ch0[:], start=False, stop=False)
                    nc.tensor.matmul(ps[:], lhsT=zh_g[:, 1, us], rhs=ch1[:], start=False, stop=False)
                    nc.tensor.matmul(ps[:], lhsT=zh_g[:, 1, us], rhs=cl1[:], start=False, stop=False)
                    nc.tensor.matmul(ps[:], lhsT=zl_g[:, 1, us], rhs=ch1[:], start=False, stop=True)

                    v = v_p.tile([P, K], f32, tag="v")
                    nc.scalar.activation(
                        out=v[:], in_=ps[:],
                        func=mybir.ActivationFunctionType.Identity,
                        bias=an_all[:, t : t + 1], scale=2.0,
                    )
                    m8 = m_p.tile([P, 8], f32, tag="m8")
                    nc.vector.max(out=m8[:], in_=v[:])
                    i8 = m_p.tile([P, 8], u32, tag="i8")
                    nc.vector.max_index(out=i8[:], in_max=m8[:], in_values=v[:])

                    # the HW SWDGE indirect gather handles one row per
                    # partition per call, so gather per tile into a slice
                    # of the per-group output buffer
                    nc.gpsimd.indirect_dma_start(
                        out=qu_g[:, u, :],
                        out_offset=None,
                        in_=cb_in[:, :],
                        in_offset=bass.IndirectOffsetOnAxis(ap=i8[:, 0:1], axis=0),
                        element_offset=b * K * D,
                    )

                nc.sync.dma_start(
                    out=q_out[b, gs, :].rearrange("(t p) d -> p t d", p=P),
                    in_=qu_g[:],
                )

    nc.compile()
    return nc


def _get_nc():
    if "nc" not in _NC_CACHE:
        _NC_CACHE["nc"] = _build()
    return _NC_CACHE["nc"]


def kernel(z_e_x, C, weight):
    global LAST_RESULT
    import ml_dtypes
    from concourse.bass_utils import run_bass_kernel_spmd

    z_e_x = np.asarray(z_e_x, dtype=np.float32)
    C = np.asarray(C).astype(np.int64)
    weight = np.asarray(weight, dtype=np.float32)

    # ||z_n||^2 computed with the exact op sequence of the reference on the
    # default jax backend, so the fp32 bits match the reference's dist term.
    import jax.numpy as jnp

    zj = jnp.asarray(z_e_x)
    zr = jnp.transpose(zj, (0, 2, 3, 1)).reshape(B, HW, D)
    A = jnp.sum(zr * zr, axis=-1, keepdims=True)
    an = -np.asarray(A)[..., 0]  # [B, HW] fp32, negated for the ACT bias

    zflat = z_e_x.reshape(B, D, HW)
    zh = zflat.astype(ml_dtypes.bfloat16)
    zl = (zflat - zh.astype(np.float32)).astype(ml_dtypes.bfloat16)

    cb_all = weight[C]  # [B, K, D] fp32
    ch = cb_all.astype(ml_dtypes.bfloat16)
    cl = (cb_all - ch.astype(np.float32)).astype(ml_dtypes.bfloat16)
    chT = np.ascontiguousarray(np.swapaxes(ch, 1, 2))  # [B, D, K] bf16
    clT = np.ascontiguousarray(np.swapaxes(cl, 1, 2))

    nc = _get_nc()
    in_maps = []
    for c in range(NCORES):
        bs = slice(c * BPC, (c + 1) * BPC)
        in_maps.append(
            dict(
                zh=np.ascontiguousarray(zh[bs]),
                zl=np.ascontiguousarray(zl[bs]),
                chT=chT[bs],
                clT=clT[bs],
                cb=np.ascontiguousarray(ch[bs].reshape(BPC * K, D)),
                an=np.ascontiguousarray(an[bs]).astype(np.float32),
            )
        )
    res = run_bass_kernel_spmd(nc, in_maps, core_ids=list(range(NCORES)), trace=TRACE)
    LAST_RESULT = res
    qr = np.concatenate([np.asarray(r["q"]) for r in res.results], 0)  # [B, HW, D] bf16
    q = np.ascontiguousarray(
        qr.astype(np.float32).reshape(B, 64, 64, D).transpose(0, 3, 1, 2)
    )
    return q, q  # (z_q_x, z_q_x_bar) - numerically identical within fp32 noise
